# revision 4
# baseline (speedup 1.0000x reference)
"""Additive (Bahdanau) attention kernel for Trainium2, SPMD over 8 NeuronCores.

Math: the reference computes
    score_e[b,e] = enc[b,e,:] @ W_enc @ w_out
    score_d[b,d] = dec[b,d,:] @ W_dec @ w_out
    attn[b,d,:]  = softmax_e(score_d[b,d] + score_e[b,:] + b_out + log mask[b,:])
    ctx[b,d,:]   = attn[b,d,:] @ enc[b]
Softmax over e is invariant to terms constant in e, so score_d and b_out cancel:
every decoder position d shares one distribution p[b,:] = softmax_e(score_e +
log mask). The outputs are p and p @ enc broadcast along d. decoder_states,
decoder_mask, W_dec and b_out never touch the result.

Sharding: data-parallel over batch, 2 batches per core, no collectives.
Per core: score_e via VectorE multiply+reduce over resident encoder tiles,
softmax partly cross-partition via GpSimd, ctx via TensorE (p as stationary),
then partition-broadcast + strided DMA to write the d-replicated outputs.
"""

import sys

for _p in ("/opt/trn_rl_repo", "/root/.axon_site/_ro/trn_rl_repo"):
    if _p not in sys.path:
        sys.path.append(_p)

from contextlib import ExitStack

import numpy as np

import concourse.bacc as bacc
import concourse.bass as bass
import concourse.bass_isa as bass_isa
import concourse.tile as tile
from concourse import mybir
from concourse.bass_utils import run_bass_kernel_spmd
from concourse.masks import make_identity

N_CORES = 8
BATCH = 16
B_PER = BATCH // N_CORES  # 2 batches per core
DEC = 256
E = 2048
C = 1024
M = 512
ET = E // 128  # 16 e-tiles per batch
CT = C // 128  # 8 c-tiles of W_enc
F32 = mybir.dt.float32

_CACHED_NC = None


def build_nc():
    nc = bacc.Bacc("TRN2", target_bir_lowering=False, debug=False, num_devices=N_CORES)

    enc = nc.declare_dram_parameter("enc", [B_PER, E, C], F32, isOutput=False)
    msk = nc.declare_dram_parameter("msk", [B_PER, E], F32, isOutput=False)
    w_enc = nc.declare_dram_parameter("w_enc", [C, M], F32, isOutput=False)
    w_out = nc.declare_dram_parameter("w_out", [1, M], F32, isOutput=False)
    attn = nc.declare_dram_parameter("attn", [B_PER, DEC, E], F32, isOutput=True)
    ctxout = nc.declare_dram_parameter("ctxout", [B_PER, DEC, C], F32, isOutput=True)

    with tile.TileContext(nc) as tc, ExitStack() as ex:
        consts = ex.enter_context(tc.tile_pool(name="consts", bufs=1))
        wpool = ex.enter_context(tc.tile_pool(name="wpool", bufs=3))
        encp = ex.enter_context(tc.tile_pool(name="encp", bufs=2 * ET // 2))
        scr = ex.enter_context(tc.tile_pool(name="scr", bufs=2))
        stats = ex.enter_context(tc.tile_pool(name="stats", bufs=2))
        rows = ex.enter_context(tc.tile_pool(name="rows", bufs=1))
        bcast = ex.enter_context(tc.tile_pool(name="bcast", bufs=1))
        psum = ex.enter_context(tc.tile_pool(name="psum", bufs=2, space="PSUM"))
        psum_ctx_pool = ex.enter_context(
            tc.tile_pool(name="psum_ctx", bufs=1, space="PSUM")
        )

        identity = consts.tile([128, 128], F32)
        make_identity(nc, identity[:, :])

        # ---- stream in the encoder shard: 8 chunks of [128, 2, 1024] per batch
        chunks = []  # chunks[b][j] covers e-tiles 2j, 2j+1 of batch b
        for b in range(B_PER):
            enc_b = enc.ap()[b].rearrange("(t p) c -> p t c", p=128)
            per_b = []
            for j in range(ET // 2):
                ch = encp.tile([128, 2, C], F32)
                nc.sync.dma_start(out=ch[:, :, :], in_=enc_b[:, 2 * j : 2 * j + 2, :])
                per_b.append(ch)
            chunks.append(per_b)

        # ---- v_enc = W_enc @ w_out, ending replicated on all partitions
        w_bc = consts.tile([128, M], F32)
        w_out_ap = w_out.ap()
        nc.sync.dma_start(
            out=w_bc[:, :],
            in_=bass.AP(
                tensor=w_out_ap.tensor,
                offset=w_out_ap.offset,
                ap=[[0, 128], w_out_ap.ap[1]],
            ),
        )
        v_stack = consts.tile([128, CT], F32)
        for t in range(CT):
            wt = wpool.tile([128, M], F32)
            nc.sync.dma_start(out=wt[:, :], in_=w_enc.ap()[t * 128 : (t + 1) * 128, :])
            junk = scr.tile([128, M], F32, tag="junk")
            nc.vector.tensor_mul(junk[:, :], wt[:, :], w_bc[:, :])
            nc.scalar.activation(
                out=junk[:, :],
                in_=junk[:, :],
                func=mybir.ActivationFunctionType.Copy,
                accum_out=v_stack[:, t : t + 1],
            )
        vT_ps_full = psum.tile([ET, 128], F32, tag="tp")
        vT_ps = vT_ps_full[:CT, :]
        nc.tensor.transpose(vT_ps[:, :], v_stack[:, :], identity[:, :])
        vT_sb = consts.tile([CT, 128], F32)
        nc.scalar.copy(out=vT_sb[:, :], in_=vT_ps[:, :])
        v_row = consts.tile([1, C], F32)
        nc.sync.dma_start(
            out=v_row[:, :].rearrange("a (t p) -> a t p", p=128), in_=vT_sb[:, :]
        )
        v_bc = consts.tile([128, C], F32)
        nc.gpsimd.partition_broadcast(v_bc[:, :], v_row[0:1, :], channels=128)

        # ---- per-batch: score -> softmax -> outputs
        for b in range(B_PER):
            # scores: s_stack[p, t] = enc[b, t*128+p, :] . v_enc
            s_stack = stats.tile([128, ET], F32)
            for t in range(ET):
                junk2 = scr.tile([128, C], F32, tag="junk2")
                nc.vector.tensor_mul(
                    junk2[:, :], chunks[b][t // 2][:, t % 2, :], v_bc[:, :]
                )
                nc.scalar.activation(
                    out=junk2[:, :],
                    in_=junk2[:, :],
                    func=mybir.ActivationFunctionType.Copy,
                    accum_out=s_stack[:, t : t + 1],
                )

            # + log(mask), transposed into the same [e%128, e//128] layout
            mask_sb = stats.tile([ET, 128], F32)
            nc.sync.dma_start(
                out=mask_sb[:, :], in_=msk.ap()[b].rearrange("(t p) -> t p", p=128)
            )
            maskT_ps = psum.tile([128, ET], F32)
            nc.tensor.transpose(maskT_ps[:, :], mask_sb[:, :], identity[:ET, :ET])
            ln_mask = stats.tile([128, ET], F32)
            nc.scalar.activation(
                out=ln_mask[:, :],
                in_=maskT_ps[:, :],
                func=mybir.ActivationFunctionType.Ln,
            )
            nc.vector.tensor_add(s_stack[:, :], s_stack[:, :], ln_mask[:, :])

            # softmax over all 2048 entries (partitions x ET)
            m_part = stats.tile([128, 1], F32)
            nc.vector.tensor_reduce(
                out=m_part[:, :],
                in_=s_stack[:, :],
                axis=mybir.AxisListType.X,
                op=mybir.AluOpType.max,
            )
            m_all = stats.tile([128, 1], F32)
            nc.gpsimd.partition_all_reduce(
                m_all[:, :], m_part[:, :], channels=128, reduce_op=bass_isa.ReduceOp.max
            )
            neg_m = stats.tile([128, 1], F32)
            nc.vector.tensor_scalar_mul(neg_m[:, :], m_all[:, :], -1.0)

            p_stack = stats.tile([128, ET], F32)
            sum_part = stats.tile([128, 1], F32)
            nc.scalar.activation(
                out=p_stack[:, :],
                in_=s_stack[:, :],
                func=mybir.ActivationFunctionType.Exp,
                bias=neg_m[:, 0:1],
                scale=1.0,
                accum_out=sum_part[:, 0:1],
            )
            sum_all = stats.tile([128, 1], F32)
            nc.gpsimd.partition_all_reduce(
                sum_all[:, :],
                sum_part[:, :],
                channels=128,
                reduce_op=bass_isa.ReduceOp.add,
            )
            inv = stats.tile([128, 1], F32)
            nc.vector.reciprocal(inv[:, :], sum_all[:, :])
            nc.vector.tensor_scalar_mul(p_stack[:, :], p_stack[:, :], inv[:, 0:1])

            # attn output: p as a row, replicated across partitions, DMA'd
            # with a stride-0 repeat over the 256 decoder positions
            pT_ps = psum.tile([ET, 128], F32, tag="tp")
            nc.tensor.transpose(pT_ps[:, :], p_stack[:, :], identity[:, :])
            pT_sb = stats.tile([ET, 128], F32)
            nc.scalar.copy(out=pT_sb[:, :], in_=pT_ps[:, :])
            p_row = rows.tile([1, E], F32)
            nc.sync.dma_start(
                out=p_row[:, :].rearrange("a (t p) -> a t p", p=128), in_=pT_sb[:, :]
            )
            p_bc = bcast.tile([128, E], F32)
            nc.gpsimd.partition_broadcast(p_bc[:, :], p_row[0:1, :], channels=128)
            p_bc_ap = p_bc[:, :]
            nc.scalar.dma_start(
                out=attn.ap()[b].rearrange("(r p) e -> p r e", p=128),
                in_=bass.AP(
                    tensor=p_bc_ap.tensor,
                    offset=p_bc_ap.offset,
                    ap=[p_bc_ap.ap[0], [0, DEC // 128], p_bc_ap.ap[1]],
                ),
            )

            # ctx = p @ enc via TensorE, p column as stationary
            ctx_ps = psum_ctx_pool.tile([1, C], F32)
            for t in range(ET):
                rhs = chunks[b][t // 2]
                for h in range(2):
                    nc.tensor.matmul(
                        ctx_ps[0:1, h * 512 : (h + 1) * 512],
                        p_stack[:, t : t + 1],
                        rhs[:, t % 2, h * 512 : (h + 1) * 512],
                        start=(t == 0),
                        stop=(t == ET - 1),
                    )
            ctx_row = rows.tile([1, C], F32)
            nc.scalar.copy(out=ctx_row[:, :], in_=ctx_ps[:, :])
            ctx_bc = bcast.tile([128, C], F32)
            nc.gpsimd.partition_broadcast(ctx_bc[:, :], ctx_row[0:1, :], channels=128)
            ctx_bc_ap = ctx_bc[:, :]
            nc.scalar.dma_start(
                out=ctxout.ap()[b].rearrange("(r p) c -> p r c", p=128),
                in_=bass.AP(
                    tensor=ctx_bc_ap.tensor,
                    offset=ctx_bc_ap.offset,
                    ap=[ctx_bc_ap.ap[0], [0, DEC // 128], ctx_bc_ap.ap[1]],
                ),
            )

    nc.compile()
    return nc


def get_nc():
    global _CACHED_NC
    if _CACHED_NC is None:
        _CACHED_NC = build_nc()
    return _CACHED_NC


def make_in_maps(encoder_states, encoder_mask, W_enc, w_out):
    enc = np.ascontiguousarray(np.asarray(encoder_states, dtype=np.float32))
    msk = np.ascontiguousarray(np.asarray(encoder_mask, dtype=np.float32))
    we = np.ascontiguousarray(np.asarray(W_enc, dtype=np.float32))
    wo = np.ascontiguousarray(np.asarray(w_out, dtype=np.float32).reshape(1, M))
    in_maps = []
    for k in range(N_CORES):
        sl = slice(k * B_PER, (k + 1) * B_PER)
        in_maps.append({"enc": enc[sl], "msk": msk[sl], "w_enc": we, "w_out": wo})
    return in_maps


def kernel(
    decoder_states,
    decoder_mask,
    encoder_states,
    encoder_mask,
    W_enc,
    W_dec,
    w_out,
    b_out,
    _want_results=False,
    _trace=False,
):
    nc = get_nc()
    in_maps = make_in_maps(encoder_states, encoder_mask, W_enc, w_out)
    res = run_bass_kernel_spmd(
        nc, in_maps, core_ids=list(range(N_CORES)), trace=_trace
    )
    context = np.concatenate([r["ctxout"] for r in res.results], axis=0)
    attn_dist = np.concatenate([r["attn"] for r in res.results], axis=0)
    if _want_results:
        return (context, attn_dist), res
    return context, attn_dist
